# revision 15
# baseline (speedup 1.0000x reference)
"""Trainium2 Bass kernel for a pre-norm transformer encoder layer (SwiGLU FFN).

Shapes (hardcoded): x [2, 2048, 768], mask [2, 2048, 2048] int32,
wq/wk/wv/wo [768, 768], w1/w3 [3072, 768], w2 [768, 3072], g_attn/g_ffn [768].

Sharding: 8 cores = 2 batch x 4 query-slices of 512 tokens. Each core
computes K/V for its full batch element (replicated within the group of 4)
and attention + FFN for its own 512 tokens. No collectives.

On-device layout is feature-major ("transposed"): activations [D, tokens].
QKV projections and the FFN w1/w3 matmuls run in fp8e4 DoubleRow (2x PE);
scores / attnV / wo / w2 run in bf16. All accumulation fp32 in PSUM.
"""
import os
import sys

for _p in ("/opt/trn_rl_repo", "/root/.axon_site/_ro/trn_rl_repo"):
    if os.path.isdir(_p) and _p not in sys.path:
        sys.path.append(_p)

import numpy as np
import ml_dtypes

import concourse.bacc as bacc
import concourse.tile as tile
from concourse import mybir

F32 = mybir.dt.float32
BF16 = mybir.dt.bfloat16
F8 = mybir.dt.float8e4
AF = mybir.ActivationFunctionType
DR = mybir.MatmulPerfMode.DoubleRow

B, S, D, H = 2, 2048, 768, 12
DK = D // H            # 64
F = 4 * D              # 3072
T = 512                # local query tokens per core
NCH = D // 128         # 6 feature chunks
NCP = NCH // 2         # 3 feature chunk-pairs (fp8 DoubleRow)
NFC = F // 128         # 24 FFN chunks
NKT = S // 128         # 16 key tiles
NQT = S // T           # 4 query slices per batch element
EPS = 1e-5
RD = 1.0 / D

# act_info.json table-set ids (see hw_specs.get_activation_tables):
#   6 = natural_log_exp_and_others (exp, ln, copy, square, identity)
ACT_SET_LNEXP = 6
W13_BUFS = 8           # fp8 FFN weight prefetch ring depth


def dr3(ap):
    """View a [128, 2*N] AP as the DoubleRow 3D form [128, 2, N]."""
    return ap.rearrange("p (j n) -> p j n", j=2)


def build_nc():
    nc = bacc.Bacc("TRN2", target_bir_lowering=False, debug=False, num_devices=8)

    xT = nc.dram_tensor("xT", [NCH, 128, S], F32, kind="ExternalInput").ap()
    maskT = nc.dram_tensor("maskT", [128, NKT * T], BF16, kind="ExternalInput").ap()
    # fp8 DoubleRow weights: [cp, p, (j, m)] with d = cp*256 + j*128 + p
    wq8 = nc.dram_tensor("wq8", [NCP, 128, 2 * D], F8, kind="ExternalInput").ap()
    wk8 = nc.dram_tensor("wk8", [NCP, 128, 2 * D], F8, kind="ExternalInput").ap()
    wv8 = nc.dram_tensor("wv8", [NCP, 128, 2 * D], F8, kind="ExternalInput").ap()
    wo8 = nc.dram_tensor("wo8", [H // 2, DK, 2 * D], BF16, kind="ExternalInput").ap()
    # [f, p, (cp, j, m)]
    w1_8 = nc.dram_tensor("w1_8", [NFC, 128, D], F8, kind="ExternalInput").ap()
    w3_8 = nc.dram_tensor("w3_8", [NFC, 128, D], F8, kind="ExternalInput").ap()
    w2_8 = nc.dram_tensor("w2_8", [NCH, 128, F], F8, kind="ExternalInput").ap()
    ones8 = nc.dram_tensor("ones8", [128, 256], F8, kind="ExternalInput").ap()
    ones16 = nc.dram_tensor("ones16", [128, 128], BF16, kind="ExternalInput").ap()

    outT = nc.dram_tensor("outT", [NCH, 128, T], F32, kind="ExternalOutput").ap()
    warm_out = nc.dram_tensor("warm_out", [2, 128, T], BF16,
                              kind="ExternalOutput").ap()

    with tile.TileContext(nc) as tc:
        with tc.tile_pool(name="glob", bufs=1) as Pg:
            # pin the exp+ln activation table once; silu triggers one switch
            nc.scalar.add_instruction(mybir.InstLoadActFuncSet(
                name=nc.get_next_instruction_name(), ins=[], outs=[],
                act_func_set_id=ACT_SET_LNEXP))

            ones16_t = Pg.tile([128, 128], BF16, name="ones16_t")
            nc.sync.dma_start(ones16_t[:], ones16)
            ones8_t = Pg.tile([128, 256], F8, name="ones8_t")
            nc.sync.dma_start(ones8_t[:], ones8)

            def warm_burst(idx, psum_pool, tag, sbuf_pool, rhs, pbufs, n=12, dve=False):
                """Dense matmul burst to trip the PE HAM to 2.4GHz."""
                wp = psum_pool.tile([128, T], F32, tag=tag, bufs=pbufs,
                                    name=f"wrm_ps{idx}")
                for i in range(n):
                    nc.tensor.matmul(wp[:], ones16_t[:], rhs,
                                     start=(i == 0), stop=(i == n - 1))
                if idx is None:
                    return
                ws = sbuf_pool.tile([128, T], BF16, tag="wrm_sb",
                                    name=f"wrm_sb{idx}")
                if dve:
                    nc.vector.tensor_copy(ws[:], wp[:])
                else:
                    nc.scalar.copy(ws[:], wp[:])
                nc.sync.dma_start(warm_out[idx], ws[:])

            eps_t = Pg.tile([128, 1], F32, name="eps_t")
            nc.vector.memset(eps_t[:], EPS)
            # xloc holds x (residual) during attention, then h in-place
            xloc = [Pg.tile([128, T], F32, name=f"xloc{c}") for c in range(NCH)]
            warm_rhs = Pg.tile([128, T], BF16, name="warm_rhs")

            # fp8 squared-h pair tiles (written in the wo loop, read by
            # the FFN mean-square matmuls)
            sqh8 = [Pg.tile([128, 2 * T], F8, name=f"sqh{p}")
                    for p in range(NCP)]
            # fp8 FFN weight prefetch rings
            w1r, w3r = {}, {}

            def w13_fetch(f):
                w1r[f] = Pg.tile([128, D], F8, tag="w1r", bufs=W13_BUFS,
                                 name=f"w1_{f}")
                nc.sync.dma_start(w1r[f][:], w1_8[f])
                w3r[f] = Pg.tile([128, D], F8, tag="w3r", bufs=W13_BUFS,
                                 name=f"w3_{f}")
                nc.sync.dma_start(w3r[f][:], w3_8[f])

            def make_phase_a(KT, QT, maskT_t, probs_pool, probs_bufs):
                def phase_a(pc, g, probs, ps_pool, ps_tag, ps_bufs):
                    """Scores + exp + mask for head pair pc, group g."""
                    heads = (2 * pc, 2 * pc + 1)
                    pss = {}
                    for h in heads:             # h-major: exp(h0) starts
                        r0 = (h % 2) * DK       # after only two matmuls
                        pss[h] = ps_pool.tile([128, 1024], F32, tag=ps_tag,
                                              bufs=ps_bufs,
                                              name=f"ps_sc{h}_{g}")
                        for j in range(2):
                            kt = 2 * g + j
                            ksl = slice(kt * 128, (kt + 1) * 128)
                            nc.tensor.matmul(
                                pss[h][:, j * T:(j + 1) * T],
                                KT[pc][r0:r0 + DK, ksl],
                                QT[pc][r0:r0 + DK, :],
                                start=True, stop=True)
                    for h in heads:
                        pr = probs_pool.tile([128, 1024], BF16, tag="probs",
                                             bufs=probs_bufs,
                                             name=f"probs{h}_{g}")
                        nc.scalar.activation(pr[:], pss[h][:], AF.Exp)
                        nc.vector.tensor_mul(
                            pr[:], pr[:],
                            maskT_t[:, g * 1024:(g + 1) * 1024])
                        probs[(h, g)] = pr
                return phase_a

            with tc.tile_pool(name="attn", bufs=1) as Pa:
                KT = [Pa.tile([128, S], BF16, name=f"KT{c}") for c in range(NCH)]
                QT = [Pa.tile([128, T], BF16, name=f"QT{c}") for c in range(NCH)]
                VA = [Pa.tile([128, H * (DK + 1)], BF16, name=f"VA{t}")
                      for t in range(NKT)]
                maskT_t = Pa.tile([128, NKT * T], BF16, name="maskT_t")

                # pair 0 runs its full attention during stage 1: probs are
                # produced and consumed within each qt slice (ring of 4)
                probs = {}
                phase_a0 = make_phase_a(KT, QT, maskT_t, Pa, 4)
                attnT = [Pa.tile([DK, 2 * T], BF16, name=f"attnT{pc}")
                         for pc in range(H // 2)]
                srows = Pa.tile([1, 2 * T], F32, name="srows")

                def phase_b(pc, g, accs, probs):
                    """attn @ V accumulation for head pair pc, group g."""
                    for h in (2 * pc, 2 * pc + 1):
                        pr = probs.pop((h, g))
                        for j in range(2):
                            kt = 2 * g + j
                            nc.tensor.matmul(
                                accs[h][0:DK + 1, :],
                                VA[kt][:, h * (DK + 1):(h + 1) * (DK + 1)],
                                pr[:, j * T:(j + 1) * T],
                                start=(g == 0 and j == 0),
                                stop=(g == 7 and j == 1))

                def pair_tail(pc, accs):
                    """1/sum + broadcast + normalized attnT for pair pc."""
                    heads = (2 * pc, 2 * pc + 1)
                    for i, h in enumerate(heads):
                        nc.vector.tensor_copy(
                            srows[0:1, i * T:(i + 1) * T],
                            accs[h][DK:DK + 1, :])
                    lnr = Pa.tile([1, 2 * T], F32, tag="lnr", bufs=2,
                                  name=f"lnr{pc}")
                    nc.scalar.activation(lnr[:], srows[:], AF.Ln)
                    srec = Pa.tile([1, 2 * T], F32, tag="srec", bufs=2,
                                   name=f"srec{pc}")
                    nc.scalar.activation(srec[:], lnr[:], AF.Exp, scale=-1.0)
                    for i, h in enumerate(heads):
                        bc = Pa.tile([DK, T], F32, tag="bc", bufs=2,
                                     name=f"bc{h}")
                        nc.gpsimd.partition_broadcast(
                            bc[:], srec[0:1, i * T:(i + 1) * T])
                        nc.vector.tensor_mul(
                            attnT[pc][:, i * T:(i + 1) * T],
                            accs[h][0:DK, :], bc[:])

                # ---------------- stage 1: rmsnorm + Q/K/V projections --------
                with (
                    tc.tile_pool(name="s1", bufs=1) as P1,
                    tc.tile_pool(name="ps1", bufs=1, space="PSUM") as PS1,
                ):
                    wq_t = [P1.tile([128, 2 * D], F8, name=f"wq{c}")
                            for c in range(NCP)]
                    wk_t = [P1.tile([128, 2 * D], F8, name=f"wk{c}")
                            for c in range(NCP)]
                    wv_t = [P1.tile([128, 2 * D], F8, name=f"wv{c}")
                            for c in range(NCP)]

                    # warm-up operands first so the HAM burst starts
                    # immediately, then x slices + weights
                    nc.sync.dma_start(warm_rhs[:], maskT[:, 0:T])
                    xq_tiles = {}
                    for qt in range(2):
                        xq_tiles[qt] = [P1.tile([128, T], F32, name=f"xq{qt}_{c}",
                                                tag=f"xq{c}", bufs=2)
                                        for c in range(NCH)]
                        for c in range(NCH):
                            nc.sync.dma_start(xq_tiles[qt][c][:],
                                              xT[c][:, qt * T:(qt + 1) * T])
                    for c in range(NCP):
                        nc.sync.dma_start(wq_t[c][:], wq8[c])
                        nc.sync.dma_start(wk_t[c][:], wk8[c])
                        nc.sync.dma_start(wv_t[c][:], wv8[c])
                    nc.sync.dma_start(maskT_t[:], maskT)
                    warm_burst(0, PS1, "ps_k", P1, warm_rhs[:], 1, n=20)

                    def rms_head(qt):
                        """Square + mean-square matmul for slice qt."""
                        xq = xq_tiles[qt]
                        sq8 = [P1.tile([128, 2 * T], F8, name=f"sq{qt}_{p}",
                                       tag=f"sq{p}", bufs=2) for p in range(NCP)]
                        for c in range(NCH):
                            nc.scalar.activation(
                                sq8[c // 2][:, (c % 2) * T:(c % 2 + 1) * T],
                                xq[c][:], AF.Square)
                        ps_ms = PS1.tile([128, T], F32, tag="ps_ms", bufs=1,
                                         name=f"ps_ms{qt}")
                        for p in range(NCP):
                            nc.tensor.matmul(ps_ms[:], dr3(ones8_t[:]),
                                             dr3(sq8[p][:]), start=(p == 0),
                                             stop=(p == NCP - 1), perf_mode=DR)
                        return ps_ms

                    def rms_tail(qt, ps_ms):
                        """rstd + fp8 normalized activations for slice qt."""
                        xq = xq_tiles[qt]
                        lntmp = P1.tile([128, T], F32, tag="lntmp", bufs=2,
                                        name=f"ln{qt}")
                        nc.scalar.activation(lntmp[:], ps_ms[:], AF.Ln,
                                             bias=eps_t[:], scale=RD)
                        rstd = P1.tile([128, T], F32, tag="rstd", bufs=2,
                                       name=f"rstd{qt}")
                        nc.scalar.activation(rstd[:], lntmp[:], AF.Exp,
                                             scale=-0.5)
                        xn8 = [P1.tile([128, 2 * T], F8, name=f"xn{qt}_{p}",
                                       tag=f"xn{p}", bufs=2) for p in range(NCP)]
                        for c in range(NCH):
                            nc.vector.tensor_mul(
                                xn8[c // 2][:, (c % 2) * T:(c % 2 + 1) * T],
                                xq[c][:], rstd[:])
                        return xn8

                    accs0 = {h: PS1.tile([128, T], F32, tag="acc0",
                                          bufs=2, name=f"acc0_{h}")
                             for h in (0, 1)}
                    ps_ms = rms_head(0)
                    xn8 = rms_tail(0, ps_ms)

                    for qt in range(NQT):
                        sl = slice(qt * T, (qt + 1) * T)
                        local = (qt == 0)
                        if local:
                            for c in range(NCH):
                                nc.vector.tensor_copy(xloc[c][:],
                                                      xq_tiles[0][c][:])
                        # K (and local Q) projections: fp8 DoubleRow,
                        # with the V projections interleaved so the PSUM->VA
                        # evacuations hide under K matmuls
                        def kq_proj(do):
                            dsl = slice(do * 128, (do + 1) * 128)
                            ps_k = PS1.tile([128, T], F32, tag="ps_k", bufs=1,
                                            name=f"ps_k{qt}_{do}")
                            for p in range(NCP):
                                nc.tensor.matmul(
                                    ps_k[:], dr3(wk_t[p][:])[:, :, dsl],
                                    dr3(xn8[p][:]), start=(p == 0),
                                    stop=(p == NCP - 1), perf_mode=DR)
                            nc.scalar.copy(KT[do][:, sl], ps_k[:])
                            if local:
                                ps_q = PS1.tile([128, T], F32, tag="ps_k",
                                                bufs=1, name=f"ps_q{do}")
                                for p in range(NCP):
                                    nc.tensor.matmul(
                                        ps_q[:], dr3(wq_t[p][:])[:, :, dsl],
                                        dr3(xn8[p][:]), start=(p == 0),
                                        stop=(p == NCP - 1), perf_mode=DR)
                                nc.scalar.mul(QT[do][:], ps_q[:],
                                              1.0 / np.sqrt(DK))

                        def v_proj(tt):
                            gt = qt * 4 + tt
                            tsl = slice(tt * 128, (tt + 1) * 128)
                            ps_v = PS1.tile([128, D], F32, tag="ps_v", bufs=1,
                                            name=f"ps_v{gt}")
                            for p in range(NCP):
                                nc.tensor.matmul(
                                    ps_v[:, 0:512], dr3(xn8[p][:])[:, :, tsl],
                                    dr3(wv_t[p][:])[:, :, 0:512],
                                    start=(p == 0), stop=(p == NCP - 1),
                                    perf_mode=DR)
                                nc.tensor.matmul(
                                    ps_v[:, 512:768], dr3(xn8[p][:])[:, :, tsl],
                                    dr3(wv_t[p][:])[:, :, 512:768],
                                    start=(p == 0), stop=(p == NCP - 1),
                                    perf_mode=DR)
                            nc.gpsimd.memset(VA[gt][:], 1.0)
                            nc.vector.tensor_copy(
                                VA[gt][:].rearrange("p (h e) -> p h e",
                                                    e=DK + 1)[:, :, 0:DK],
                                ps_v[:].rearrange("p (h d) -> p h d", d=DK))

                        kq_proj(0)
                        kq_proj(1)
                        v_proj(0)
                        kq_proj(2)
                        kq_proj(3)
                        v_proj(1)
                        kq_proj(4)
                        kq_proj(5)
                        # prefetch + rmsnorm for the next slice; the last two
                        # V projections then hide the rstd/xn8 chain
                        nxt = qt + 1
                        if nxt < NQT:
                            if nxt + 1 < NQT:
                                xq_tiles[nxt + 1] = [
                                    P1.tile([128, T], F32, tag=f"xq{c}", bufs=2,
                                            name=f"xq{nxt + 1}_{c}")
                                    for c in range(NCH)]
                                for c in range(NCH):
                                    nc.sync.dma_start(
                                        xq_tiles[nxt + 1][c][:],
                                        xT[c][:, (nxt + 1) * T:(nxt + 2) * T])
                            ps_ms_n = rms_head(nxt)
                            xn8_n = rms_tail(nxt, ps_ms_n)
                        v_proj(2)
                        v_proj(3)
                        # pair-0 attention for the key tiles this slice just
                        # produced -- hides its exp work under stage-1 matmuls
                        for g in (2 * qt, 2 * qt + 1):
                            phase_a0(0, g, probs, PS1, "ps_sc0", 1)
                            phase_b(0, g, accs0, probs)
                        if nxt < NQT:
                            xn8 = xn8_n
                    pair_tail(0, accs0)
                    for f in range(W13_BUFS):
                        w13_fetch(f)

                # ---------------- stage 2: attention ------------------------
                with (
                    tc.tile_pool(name="s2", bufs=1) as P2,
                    tc.tile_pool(name="ps2", bufs=1, space="PSUM") as PS2,
                ):
                    wo_t = [P2.tile([DK, 2 * D], BF16, name=f"wo{pc}")
                            for pc in range(H // 2)]
                    for pc in range(H // 2):
                        nc.sync.dma_start(wo_t[pc][:], wo8[pc])
                    phase_a2 = make_phase_a(KT, QT, maskT_t, P2, 16)

                    # pair 0 completed during stage 1; pairs 1..5 here
                    for pc in range(1, H // 2):
                        heads = (2 * pc, 2 * pc + 1)
                        accs = {h: PS2.tile([128, T], F32, tag="acc", bufs=2,
                                            name=f"acc{h}") for h in heads}
                        for g in range(8):
                            phase_a2(pc, g, probs, PS2, "ps_sc", 3)
                        for g in range(8):
                            phase_b(pc, g, accs, probs)
                        pair_tail(pc, accs)
                        # keep-alive: prevent HAM idle-flip at pair boundary
                        warm_burst(None, PS2, "ps_sc", P2, warm_rhs[:], 3,
                                   n=4, dve=True)

                    # wo projection + residual (in place into xloc -> h)
                    for do in range(NCH):
                        dsl = slice(do * 128, (do + 1) * 128)
                        ps_h2 = PS2.tile([128, T], F32, tag="ps_sc", bufs=3,
                                         name=f"ps_h2_{do}")
                        for h in range(H):
                            pc, i = h // 2, h % 2
                            nc.tensor.matmul(
                                ps_h2[:],
                                dr3(wo_t[pc][:])[:, i, dsl],
                                attnT[pc][:, i * T:(i + 1) * T],
                                start=(h == 0), stop=(h == H - 1))
                        nc.vector.tensor_add(xloc[do][:], ps_h2[:], xloc[do][:])
                        nc.scalar.activation(
                            sqh8[do // 2][:, (do % 2) * T:(do % 2 + 1) * T],
                            xloc[do][:], AF.Square)

            # ------------- stage 3+4: FFN (SwiGLU) --------------------------
            with (
                tc.tile_pool(name="s4", bufs=1) as P4,
                tc.tile_pool(name="ps4", bufs=1, space="PSUM") as PS4,
            ):
                # rmsnorm mean-square (sqh8 computed inside the wo loop)
                ps_ms2 = PS4.tile([128, T], F32, tag="ps_u", bufs=3,
                                  name="ps_ms2")
                for p in range(NCP):
                    nc.tensor.matmul(ps_ms2[:], dr3(ones8_t[:]),
                                     dr3(sqh8[p][:]), start=(p == 0),
                                     stop=(p == NCP - 1), perf_mode=DR)
                lntmp2 = P4.tile([128, T], F32, name="lntmp2")
                nc.scalar.activation(lntmp2[:], ps_ms2[:], AF.Ln,
                                     bias=eps_t[:], scale=RD)
                rstd2 = P4.tile([128, T], F32, name="rstd2")
                nc.scalar.activation(rstd2[:], lntmp2[:], AF.Exp, scale=-0.5)
                hn8 = [P4.tile([128, 2 * T], F8, name=f"hn{p}")
                       for p in range(NCP)]
                for c in range(NCH):
                    nc.vector.tensor_mul(
                        hn8[c // 2][:, (c % 2) * T:(c % 2 + 1) * T],
                        xloc[c][:], rstd2[:])

                w2_t = {}
                for do in range(2):
                    w2_t[do] = P4.tile([128, F], F8, tag="w2_t", bufs=2,
                                       name=f"w2_{do}")
                    nc.sync.dma_start(w2_t[do][:], w2_8[do])

                # fp8 pair tiles: f-even in cols 0:T, f-odd in T:2T
                prod8 = [P4.tile([128, 2 * T], F8, name=f"prod{fp}")
                         for fp in range(NFC // 2)]
                for f in range(NFC):
                    if f + W13_BUFS < NFC:
                        w13_fetch(f + W13_BUFS)
                    ps_u = PS4.tile([128, T], F32, tag="ps_u", bufs=3,
                                    name=f"ps_u{f}")
                    ps_w = PS4.tile([128, T], F32, tag="ps_w", bufs=3,
                                    name=f"ps_w{f}")
                    for p in range(NCP):
                        csl = slice(p * 256, (p + 1) * 256)
                        nc.tensor.matmul(
                            ps_u[:], dr3(w1r[f][:, csl]), dr3(hn8[p][:]),
                            start=(p == 0), stop=(p == NCP - 1), perf_mode=DR)
                        nc.tensor.matmul(
                            ps_w[:], dr3(w3r[f][:, csl]), dr3(hn8[p][:]),
                            start=(p == 0), stop=(p == NCP - 1), perf_mode=DR)
                    silu = P4.tile([128, T], BF16, tag="silu", bufs=2,
                                   name=f"silu{f}")
                    if os.environ.get("BASS_SIM_SILU") == "1":
                        # CoreSim has no Silu; emulate as u*sigmoid(u)
                        nc.scalar.activation(silu[:], ps_u[:], AF.Sigmoid)
                        nc.vector.tensor_mul(silu[:], silu[:], ps_u[:])
                    else:
                        nc.scalar.activation(silu[:], ps_u[:], AF.Silu)
                    nc.vector.tensor_mul(
                        prod8[f // 2][:, (f % 2) * T:(f % 2 + 1) * T],
                        silu[:], ps_w[:])

                for do in range(NCH):
                    if do + 2 < NCH:
                        w2_t[do + 2] = P4.tile([128, F], F8, tag="w2_t",
                                               bufs=2, name=f"w2_{do + 2}")
                        nc.sync.dma_start(w2_t[do + 2][:], w2_8[do + 2])
                    ps_y = PS4.tile([128, T], F32, tag="ps_y", bufs=2,
                                    name=f"ps_y{do}")
                    for fp in range(NFC // 2):
                        fsl = slice(fp * 256, (fp + 1) * 256)
                        nc.tensor.matmul(ps_y[:], dr3(w2_t[do][:, fsl]),
                                         dr3(prod8[fp][:]), start=(fp == 0),
                                         stop=(fp == NFC // 2 - 1),
                                         perf_mode=DR)
                    outt = P4.tile([128, T], F32, tag="outt", bufs=2,
                                   name=f"outt{do}")
                    nc.vector.tensor_add(outt[:], ps_y[:], xloc[do][:])
                    nc.sync.dma_start(outT[do], outt[:])

    nc.compile()
    return nc


def _f8(a):
    return np.clip(a, -240.0, 240.0).astype(ml_dtypes.float8_e4m3)


def prep_inputs(x, mask, wq, wk, wv, wo, w1, w2, w3, g_attn, g_ffn):
    """Build the 8 per-core input maps (host-side sharding + layout)."""
    bf = ml_dtypes.bfloat16

    def dr_w(w, g):
        # [cp, p, (j, m)]: lhsT[p, j, m] = (w*g)[m, cp*256 + j*128 + p]
        wt = (w * g[None, :]).T                      # [d_in, d_out]
        return _f8(np.ascontiguousarray(
            wt.reshape(NCP, 2, 128, D).transpose(0, 2, 1, 3)
            .reshape(NCP, 128, 2 * D)))

    wq8 = dr_w(wq, g_attn)
    wk8 = dr_w(wk, g_attn)
    wv8 = dr_w(wv, g_attn)
    # wo8[pc, p, j, m] = wo[m, (2pc+j)*64 + p]
    wo8 = np.ascontiguousarray(
        wo.T.reshape(H // 2, 2, DK, D).transpose(0, 2, 1, 3)
        .reshape(H // 2, DK, 2 * D)).astype(bf)

    def dr_ffn(w, g):
        # [f, p, (cp, j, m)]: lhsT[p, cp, j, m] = (w*g)[f*128+m, cp*256+j*128+p]
        wt = (w * g[None, :]).T                      # [D, F]
        t = wt.reshape(NCP, 2, 128, NFC, 128).transpose(3, 2, 0, 1, 4)
        return _f8(np.ascontiguousarray(t.reshape(NFC, 128, D)))

    w1_8 = dr_ffn(w1, g_ffn)
    w3_8 = dr_ffn(w3, g_ffn)
    # w2_8[do, p, fp, j, m] = w2[do*128+m, fp*256 + j*128 + p]
    w2_8v = _f8(np.ascontiguousarray(
        w2.T.reshape(NFC // 2, 2, 128, NCH, 128).transpose(3, 2, 0, 1, 4)
        .reshape(NCH, 128, F)))
    ones8 = np.ones((128, 256), ml_dtypes.float8_e4m3)
    ones16 = np.ones((128, 128), bf)

    in_maps = []
    for core in range(8):
        b, qt = core // NQT, core % NQT
        # rotate tokens so the local 512-query slice is always quarter 0
        order = (np.arange(S) + qt * T) % S
        xb = x[b][order]                       # [S, D] rotated
        xTe = np.ascontiguousarray(xb.T.reshape(NCH, 128, S)).astype(np.float32)
        # maskT[p, kt*T + q] = mask[b, qt*T + q, k] with k = kt*128 + p in
        # ROTATED key order (keys follow the same rotation as tokens).
        msl = mask[b, qt * T:(qt + 1) * T][:, order]     # [T(q), S(k)] rotated
        maskTe = np.ascontiguousarray(
            msl.T.reshape(NKT, 128, T).transpose(1, 0, 2)
            .reshape(128, NKT * T)).astype(bf)
        in_maps.append({
            "xT": xTe, "maskT": maskTe,
            "wq8": wq8, "wk8": wk8, "wv8": wv8, "wo8": wo8,
            "w1_8": w1_8, "w3_8": w3_8, "w2_8": w2_8v,
            "ones8": ones8, "ones16": ones16,
        })
    return in_maps


_NC_CACHE = None


def get_nc():
    global _NC_CACHE
    if _NC_CACHE is None:
        _NC_CACHE = build_nc()
    return _NC_CACHE


def gather_output(results):
    out = np.empty((B, S, D), np.float32)
    for core in range(8):
        b, qt = core // NQT, core % NQT
        o = results[core]["outT"]              # [NCH, 128, T]
        out[b, qt * T:(qt + 1) * T, :] = o.reshape(D, T).T
    return out


def kernel(**inputs):
    from concourse.bass_utils import run_bass_kernel_spmd
    in_maps = prep_inputs(
        np.asarray(inputs["x"]), np.asarray(inputs["mask"]),
        np.asarray(inputs["wq"]), np.asarray(inputs["wk"]),
        np.asarray(inputs["wv"]), np.asarray(inputs["wo"]),
        np.asarray(inputs["w1"]), np.asarray(inputs["w2"]),
        np.asarray(inputs["w3"]),
        np.asarray(inputs["g_attn"]), np.asarray(inputs["g_ffn"]))
    nc = get_nc()
    res = run_bass_kernel_spmd(nc, in_maps, core_ids=list(range(8)))
    return gather_output(res.results)


# revision 17
# speedup vs baseline: 1.0258x; 1.0258x over previous
"""Trainium2 Bass kernel for a pre-norm transformer encoder layer (SwiGLU FFN).

Shapes (hardcoded): x [2, 2048, 768], mask [2, 2048, 2048] int32,
wq/wk/wv/wo [768, 768], w1/w3 [3072, 768], w2 [768, 3072], g_attn/g_ffn [768].

Sharding: 8 cores = 2 batch x 4 query-slices of 512 tokens. Each core
computes K/V for its full batch element (replicated within the group of 4)
and attention + FFN for its own 512 tokens. No collectives.

On-device layout is feature-major ("transposed"): activations [D, tokens].
QKV projections and the FFN w1/w3 matmuls run in fp8e4 DoubleRow (2x PE);
scores / attnV / wo / w2 run in bf16. All accumulation fp32 in PSUM.
"""
import os
import sys

for _p in ("/opt/trn_rl_repo", "/root/.axon_site/_ro/trn_rl_repo"):
    if os.path.isdir(_p) and _p not in sys.path:
        sys.path.append(_p)

import numpy as np
import ml_dtypes

import concourse.bacc as bacc
import concourse.tile as tile
from concourse import mybir

F32 = mybir.dt.float32
BF16 = mybir.dt.bfloat16
F8 = mybir.dt.float8e4
AF = mybir.ActivationFunctionType
DR = mybir.MatmulPerfMode.DoubleRow

B, S, D, H = 2, 2048, 768, 12
DK = D // H            # 64
F = 4 * D              # 3072
T = 512                # local query tokens per core
NCH = D // 128         # 6 feature chunks
NCP = NCH // 2         # 3 feature chunk-pairs (fp8 DoubleRow)
NFC = F // 128         # 24 FFN chunks
NKT = S // 128         # 16 key tiles
NQT = S // T           # 4 query slices per batch element
EPS = 1e-5
RD = 1.0 / D

# act_info.json table-set ids (see hw_specs.get_activation_tables):
#   6 = natural_log_exp_and_others (exp, ln, copy, square, identity)
ACT_SET_LNEXP = 6
W13_BUFS = 8           # fp8 FFN weight prefetch ring depth


def dr3(ap):
    """View a [128, 2*N] AP as the DoubleRow 3D form [128, 2, N]."""
    return ap.rearrange("p (j n) -> p j n", j=2)


def build_nc():
    nc = bacc.Bacc("TRN2", target_bir_lowering=False, debug=False, num_devices=8)

    xT = nc.dram_tensor("xT", [NCH, 128, S], F32, kind="ExternalInput").ap()
    maskT = nc.dram_tensor("maskT", [128, NKT * T], BF16, kind="ExternalInput").ap()
    # fp8 DoubleRow weights: [cp, p, (j, m)] with d = cp*256 + j*128 + p
    wq8 = nc.dram_tensor("wq8", [NCP, 128, 2 * D], F8, kind="ExternalInput").ap()
    wk8 = nc.dram_tensor("wk8", [NCP, 128, 2 * D], F8, kind="ExternalInput").ap()
    wv8 = nc.dram_tensor("wv8", [NCP, 128, 2 * D], F8, kind="ExternalInput").ap()
    wo8 = nc.dram_tensor("wo8", [H // 2, DK, 2 * D], BF16, kind="ExternalInput").ap()
    # [f, p, (cp, j, m)]
    w1_8 = nc.dram_tensor("w1_8", [NFC, 128, D], F8, kind="ExternalInput").ap()
    w3_8 = nc.dram_tensor("w3_8", [NFC, 128, D], F8, kind="ExternalInput").ap()
    w2_8 = nc.dram_tensor("w2_8", [NCH, 128, F], F8, kind="ExternalInput").ap()
    ones8 = nc.dram_tensor("ones8", [128, 256], F8, kind="ExternalInput").ap()
    ones16 = nc.dram_tensor("ones16", [128, 128], BF16, kind="ExternalInput").ap()

    outT = nc.dram_tensor("outT", [NCH, 128, T], F32, kind="ExternalOutput").ap()
    warm_out = nc.dram_tensor("warm_out", [2, 128, T], BF16,
                              kind="ExternalOutput").ap()

    with tile.TileContext(nc) as tc:
        with tc.tile_pool(name="glob", bufs=1) as Pg:
            # pin the exp+ln activation table once; silu triggers one switch
            nc.scalar.add_instruction(mybir.InstLoadActFuncSet(
                name=nc.get_next_instruction_name(), ins=[], outs=[],
                act_func_set_id=ACT_SET_LNEXP))

            ones16_t = Pg.tile([128, 128], BF16, name="ones16_t")
            nc.sync.dma_start(ones16_t[:], ones16)
            ones8_t = Pg.tile([128, 256], F8, name="ones8_t")
            nc.sync.dma_start(ones8_t[:], ones8)

            def warm_burst(idx, psum_pool, tag, sbuf_pool, rhs, pbufs, n=12, dve=False):
                """Dense matmul burst to trip the PE HAM to 2.4GHz."""
                wp = psum_pool.tile([128, T], F32, tag=tag, bufs=pbufs,
                                    name=f"wrm_ps{idx}")
                for i in range(n):
                    nc.tensor.matmul(wp[:], ones16_t[:], rhs,
                                     start=(i == 0), stop=(i == n - 1))
                if idx is None:
                    return
                ws = sbuf_pool.tile([128, T], BF16, tag="wrm_sb",
                                    name=f"wrm_sb{idx}")
                if dve:
                    nc.vector.tensor_copy(ws[:], wp[:])
                else:
                    nc.scalar.copy(ws[:], wp[:])
                nc.sync.dma_start(warm_out[idx], ws[:])

            eps_t = Pg.tile([128, 1], F32, name="eps_t")
            nc.vector.memset(eps_t[:], EPS)
            # xloc holds x (residual) during attention, then h in-place
            xloc = [Pg.tile([128, T], F32, name=f"xloc{c}") for c in range(NCH)]
            warm_rhs = Pg.tile([128, T], BF16, name="warm_rhs")

            # fp8 squared-h pair tiles (written in the wo loop, read by
            # the FFN mean-square matmuls)
            sqh8 = [Pg.tile([128, 2 * T], F8, name=f"sqh{p}")
                    for p in range(NCP)]
            # fp8 FFN weight prefetch rings
            w1r, w3r = {}, {}

            def w13_fetch(f):
                w1r[f] = Pg.tile([128, D], F8, tag="w1r", bufs=W13_BUFS,
                                 name=f"w1_{f}")
                nc.sync.dma_start(w1r[f][:], w1_8[f])
                w3r[f] = Pg.tile([128, D], F8, tag="w3r", bufs=W13_BUFS,
                                 name=f"w3_{f}")
                nc.sync.dma_start(w3r[f][:], w3_8[f])

            with tc.tile_pool(name="attn", bufs=1) as Pa:
                KT = [Pa.tile([128, S], BF16, name=f"KT{c}") for c in range(NCH)]
                QT = [Pa.tile([128, T], BF16, name=f"QT{c}") for c in range(NCH)]
                VA = [Pa.tile([128, H * (DK + 1)], BF16, name=f"VA{t}")
                      for t in range(NKT)]
                maskT_t = Pa.tile([128, NKT * T], BF16, name="maskT_t")

                # ---------------- stage 1: rmsnorm + Q/K/V projections --------
                with (
                    tc.tile_pool(name="s1", bufs=1) as P1,
                    tc.tile_pool(name="ps1", bufs=1, space="PSUM") as PS1,
                ):
                    wq_t = [P1.tile([128, 2 * D], F8, name=f"wq{c}")
                            for c in range(NCP)]
                    wk_t = [P1.tile([128, 2 * D], F8, name=f"wk{c}")
                            for c in range(NCP)]
                    wv_t = [P1.tile([128, 2 * D], F8, name=f"wv{c}")
                            for c in range(NCP)]

                    # warm-up operands first so the HAM burst starts
                    # immediately, then x slices + weights
                    nc.sync.dma_start(warm_rhs[:], maskT[:, 0:T])
                    xq_tiles = {}
                    for qt in range(2):
                        xq_tiles[qt] = [P1.tile([128, T], F32, name=f"xq{qt}_{c}",
                                                tag=f"xq{c}", bufs=2)
                                        for c in range(NCH)]
                        for c in range(NCH):
                            nc.sync.dma_start(xq_tiles[qt][c][:],
                                              xT[c][:, qt * T:(qt + 1) * T])
                    for c in range(NCP):
                        nc.sync.dma_start(wq_t[c][:], wq8[c])
                        nc.sync.dma_start(wk_t[c][:], wk8[c])
                        nc.sync.dma_start(wv_t[c][:], wv8[c])
                    nc.sync.dma_start(maskT_t[:], maskT)
                    warm_burst(0, PS1, "ps_k", P1, warm_rhs[:], 2, n=20)

                    def rms_head(qt):
                        """Square + mean-square matmul for slice qt."""
                        xq = xq_tiles[qt]
                        sq8 = [P1.tile([128, 2 * T], F8, name=f"sq{qt}_{p}",
                                       tag=f"sq{p}", bufs=2) for p in range(NCP)]
                        for c in range(NCH):
                            nc.scalar.activation(
                                sq8[c // 2][:, (c % 2) * T:(c % 2 + 1) * T],
                                xq[c][:], AF.Square)
                        ps_ms = PS1.tile([128, T], F32, tag="ps_ms", bufs=2,
                                         name=f"ps_ms{qt}")
                        for p in range(NCP):
                            nc.tensor.matmul(ps_ms[:], dr3(ones8_t[:]),
                                             dr3(sq8[p][:]), start=(p == 0),
                                             stop=(p == NCP - 1), perf_mode=DR)
                        return ps_ms

                    def rms_tail(qt, ps_ms):
                        """rstd + fp8 normalized activations for slice qt."""
                        xq = xq_tiles[qt]
                        lntmp = P1.tile([128, T], F32, tag="lntmp", bufs=2,
                                        name=f"ln{qt}")
                        nc.scalar.activation(lntmp[:], ps_ms[:], AF.Ln,
                                             bias=eps_t[:], scale=RD)
                        rstd = P1.tile([128, T], F32, tag="rstd", bufs=2,
                                       name=f"rstd{qt}")
                        nc.scalar.activation(rstd[:], lntmp[:], AF.Exp,
                                             scale=-0.5)
                        xn8 = [P1.tile([128, 2 * T], F8, name=f"xn{qt}_{p}",
                                       tag=f"xn{p}", bufs=2) for p in range(NCP)]
                        for c in range(NCH):
                            nc.vector.tensor_mul(
                                xn8[c // 2][:, (c % 2) * T:(c % 2 + 1) * T],
                                xq[c][:], rstd[:])
                        return xn8

                    ps_ms = rms_head(0)
                    xn8 = rms_tail(0, ps_ms)

                    for qt in range(NQT):
                        sl = slice(qt * T, (qt + 1) * T)
                        local = (qt == 0)
                        if local:
                            for c in range(NCH):
                                nc.vector.tensor_copy(xloc[c][:],
                                                      xq_tiles[0][c][:])
                        # K (and local Q) projections: fp8 DoubleRow,
                        # with the V projections interleaved so the PSUM->VA
                        # evacuations hide under K matmuls
                        def kq_proj(do):
                            dsl = slice(do * 128, (do + 1) * 128)
                            ps_k = PS1.tile([128, T], F32, tag="ps_k", bufs=2,
                                            name=f"ps_k{qt}_{do}")
                            for p in range(NCP):
                                nc.tensor.matmul(
                                    ps_k[:], dr3(wk_t[p][:])[:, :, dsl],
                                    dr3(xn8[p][:]), start=(p == 0),
                                    stop=(p == NCP - 1), perf_mode=DR)
                            nc.scalar.copy(KT[do][:, sl], ps_k[:])
                            if local:
                                ps_q = PS1.tile([128, T], F32, tag="ps_k",
                                                bufs=2, name=f"ps_q{do}")
                                for p in range(NCP):
                                    nc.tensor.matmul(
                                        ps_q[:], dr3(wq_t[p][:])[:, :, dsl],
                                        dr3(xn8[p][:]), start=(p == 0),
                                        stop=(p == NCP - 1), perf_mode=DR)
                                nc.scalar.mul(QT[do][:], ps_q[:],
                                              1.0 / np.sqrt(DK))

                        def v_proj(tt):
                            gt = qt * 4 + tt
                            tsl = slice(tt * 128, (tt + 1) * 128)
                            ps_v = PS1.tile([128, D], F32, tag="ps_v", bufs=2,
                                            name=f"ps_v{gt}")
                            for p in range(NCP):
                                nc.tensor.matmul(
                                    ps_v[:, 0:512], dr3(xn8[p][:])[:, :, tsl],
                                    dr3(wv_t[p][:])[:, :, 0:512],
                                    start=(p == 0), stop=(p == NCP - 1),
                                    perf_mode=DR)
                                nc.tensor.matmul(
                                    ps_v[:, 512:768], dr3(xn8[p][:])[:, :, tsl],
                                    dr3(wv_t[p][:])[:, :, 512:768],
                                    start=(p == 0), stop=(p == NCP - 1),
                                    perf_mode=DR)
                            nc.gpsimd.memset(VA[gt][:], 1.0)
                            nc.vector.tensor_copy(
                                VA[gt][:].rearrange("p (h e) -> p h e",
                                                    e=DK + 1)[:, :, 0:DK],
                                ps_v[:].rearrange("p (h d) -> p h d", d=DK))

                        kq_proj(0)
                        kq_proj(1)
                        v_proj(0)
                        kq_proj(2)
                        kq_proj(3)
                        v_proj(1)
                        kq_proj(4)
                        kq_proj(5)
                        # prefetch + rmsnorm for the next slice; the last two
                        # V projections then hide the rstd/xn8 chain
                        nxt = qt + 1
                        if nxt < NQT:
                            if nxt + 1 < NQT:
                                xq_tiles[nxt + 1] = [
                                    P1.tile([128, T], F32, tag=f"xq{c}", bufs=2,
                                            name=f"xq{nxt + 1}_{c}")
                                    for c in range(NCH)]
                                for c in range(NCH):
                                    nc.sync.dma_start(
                                        xq_tiles[nxt + 1][c][:],
                                        xT[c][:, (nxt + 1) * T:(nxt + 2) * T])
                            ps_ms_n = rms_head(nxt)
                            xn8_n = rms_tail(nxt, ps_ms_n)
                        v_proj(2)
                        v_proj(3)
                        if nxt < NQT:
                            xn8 = xn8_n
                    for f in range(W13_BUFS):
                        w13_fetch(f)

                # ---------------- stage 2: attention ------------------------
                with (
                    tc.tile_pool(name="s2", bufs=1) as P2,
                    tc.tile_pool(name="ps2", bufs=1, space="PSUM") as PS2,
                ):
                    wo_t = [P2.tile([DK, 2 * D], BF16, name=f"wo{pc}")
                            for pc in range(H // 2)]
                    for pc in range(H // 2):
                        nc.sync.dma_start(wo_t[pc][:], wo8[pc])
                    attnT = [P2.tile([DK, 2 * T], BF16, name=f"attnT{pc}")
                             for pc in range(H // 2)]
                    srows = P2.tile([1, 2 * T], F32, name="srows")

                    def phase_a(pc, g, probs):
                        """Scores + exp + mask for head pair pc, group g."""
                        heads = (2 * pc, 2 * pc + 1)
                        pss = {}
                        for h in heads:             # h-major: exp(h0) starts
                            r0 = (h % 2) * DK       # after only two matmuls
                            pss[h] = PS2.tile([128, 1024], F32,
                                              tag="ps_sc", bufs=3,
                                              name=f"ps_sc{h}_{g}")
                            for j in range(2):
                                kt = 2 * g + j
                                ksl = slice(kt * 128, (kt + 1) * 128)
                                nc.tensor.matmul(
                                    pss[h][:, j * T:(j + 1) * T],
                                    KT[pc][r0:r0 + DK, ksl],
                                    QT[pc][r0:r0 + DK, :],
                                    start=True, stop=True)
                        for h in heads:
                            pr = P2.tile([128, 1024], BF16, tag="probs",
                                         bufs=20, name=f"probs{h}_{g}")
                            nc.scalar.activation(pr[:], pss[h][:], AF.Exp)
                            nc.vector.tensor_mul(
                                pr[:], pr[:],
                                maskT_t[:, g * 1024:(g + 1) * 1024])
                            probs[(h, g)] = pr

                    def phase_b(pc, g, accs, probs):
                        """attn @ V accumulation for head pair pc, group g."""
                        for h in (2 * pc, 2 * pc + 1):
                            pr = probs.pop((h, g))
                            for j in range(2):
                                kt = 2 * g + j
                                nc.tensor.matmul(
                                    accs[h][0:DK + 1, :],
                                    VA[kt][:, h * (DK + 1):(h + 1) * (DK + 1)],
                                    pr[:, j * T:(j + 1) * T],
                                    start=(g == 0 and j == 0),
                                    stop=(g == 7 and j == 1))

                    def pair_tail(pc, accs):
                        """1/sum + broadcast + normalized attnT for pair pc."""
                        heads = (2 * pc, 2 * pc + 1)
                        for i, h in enumerate(heads):
                            nc.vector.tensor_copy(
                                srows[0:1, i * T:(i + 1) * T],
                                accs[h][DK:DK + 1, :])
                        lnr = P2.tile([1, 2 * T], F32, tag="lnr", bufs=2,
                                      name=f"lnr{pc}")
                        nc.scalar.activation(lnr[:], srows[:], AF.Ln)
                        srec = P2.tile([1, 2 * T], F32, tag="srec", bufs=2,
                                       name=f"srec{pc}")
                        nc.scalar.activation(srec[:], lnr[:], AF.Exp,
                                             scale=-1.0)
                        for i, h in enumerate(heads):
                            bc = P2.tile([DK, T], F32, tag="bc", bufs=2,
                                         name=f"bc{h}")
                            nc.gpsimd.partition_broadcast(
                                bc[:], srec[0:1, i * T:(i + 1) * T])
                            nc.vector.tensor_mul(
                                attnT[pc][:, i * T:(i + 1) * T],
                                accs[h][0:DK, :], bc[:])

                    probs = {}

                    def run_b(ppc):
                        paccs = {h: PS2.tile([128, T], F32, tag="acc", bufs=2,
                                             name=f"acc{h}")
                                 for h in (2 * ppc, 2 * ppc + 1)}
                        for g in range(8):
                            phase_b(ppc, g, paccs, probs)
                        pair_tail(ppc, paccs)
                        # keep-alive: prevent HAM idle-flip at pair boundary
                        warm_burst(None, PS2, "ps_sc", P2, warm_rhs[:], 3,
                                   n=4, dve=True)

                    # hoist each pair's first two score groups ahead of the
                    # previous pair's attn@V so the exp stream never waits for
                    # score matmuls stuck behind the 32 B matmuls
                    prev = None
                    for pc in range(H // 2):
                        for g in range(2):
                            phase_a(pc, g, probs)
                        if prev is not None:
                            run_b(prev)
                        for g in range(2, 8):
                            phase_a(pc, g, probs)
                        prev = pc
                    run_b(prev)

                    # wo projection + residual (in place into xloc -> h)
                    for do in range(NCH):
                        dsl = slice(do * 128, (do + 1) * 128)
                        ps_h2 = PS2.tile([128, T], F32, tag="ps_sc", bufs=3,
                                         name=f"ps_h2_{do}")
                        for h in range(H):
                            pc, i = h // 2, h % 2
                            nc.tensor.matmul(
                                ps_h2[:],
                                dr3(wo_t[pc][:])[:, i, dsl],
                                attnT[pc][:, i * T:(i + 1) * T],
                                start=(h == 0), stop=(h == H - 1))
                        nc.vector.tensor_add(xloc[do][:], ps_h2[:], xloc[do][:])
                        nc.scalar.activation(
                            sqh8[do // 2][:, (do % 2) * T:(do % 2 + 1) * T],
                            xloc[do][:], AF.Square)

            # ------------- stage 3+4: FFN (SwiGLU) --------------------------
            with (
                tc.tile_pool(name="s4", bufs=1) as P4,
                tc.tile_pool(name="ps4", bufs=1, space="PSUM") as PS4,
            ):
                # rmsnorm mean-square (sqh8 computed inside the wo loop)
                ps_ms2 = PS4.tile([128, T], F32, tag="ps_u", bufs=3,
                                  name="ps_ms2")
                for p in range(NCP):
                    nc.tensor.matmul(ps_ms2[:], dr3(ones8_t[:]),
                                     dr3(sqh8[p][:]), start=(p == 0),
                                     stop=(p == NCP - 1), perf_mode=DR)
                lntmp2 = P4.tile([128, T], F32, name="lntmp2")
                nc.scalar.activation(lntmp2[:], ps_ms2[:], AF.Ln,
                                     bias=eps_t[:], scale=RD)
                rstd2 = P4.tile([128, T], F32, name="rstd2")
                nc.scalar.activation(rstd2[:], lntmp2[:], AF.Exp, scale=-0.5)
                hn8 = [P4.tile([128, 2 * T], F8, name=f"hn{p}")
                       for p in range(NCP)]
                for c in range(NCH):
                    nc.vector.tensor_mul(
                        hn8[c // 2][:, (c % 2) * T:(c % 2 + 1) * T],
                        xloc[c][:], rstd2[:])

                w2_t = {}
                for do in range(2):
                    w2_t[do] = P4.tile([128, F], F8, tag="w2_t", bufs=2,
                                       name=f"w2_{do}")
                    nc.sync.dma_start(w2_t[do][:], w2_8[do])

                # fp8 pair tiles: f-even in cols 0:T, f-odd in T:2T
                prod8 = [P4.tile([128, 2 * T], F8, name=f"prod{fp}")
                         for fp in range(NFC // 2)]
                for f in range(NFC):
                    if f + W13_BUFS < NFC:
                        w13_fetch(f + W13_BUFS)
                    ps_u = PS4.tile([128, T], F32, tag="ps_u", bufs=3,
                                    name=f"ps_u{f}")
                    ps_w = PS4.tile([128, T], F32, tag="ps_w", bufs=3,
                                    name=f"ps_w{f}")
                    for p in range(NCP):
                        csl = slice(p * 256, (p + 1) * 256)
                        nc.tensor.matmul(
                            ps_u[:], dr3(w1r[f][:, csl]), dr3(hn8[p][:]),
                            start=(p == 0), stop=(p == NCP - 1), perf_mode=DR)
                        nc.tensor.matmul(
                            ps_w[:], dr3(w3r[f][:, csl]), dr3(hn8[p][:]),
                            start=(p == 0), stop=(p == NCP - 1), perf_mode=DR)
                    silu = P4.tile([128, T], BF16, tag="silu", bufs=2,
                                   name=f"silu{f}")
                    if os.environ.get("BASS_SIM_SILU") == "1":
                        # CoreSim has no Silu; emulate as u*sigmoid(u)
                        nc.scalar.activation(silu[:], ps_u[:], AF.Sigmoid)
                        nc.vector.tensor_mul(silu[:], silu[:], ps_u[:])
                    else:
                        nc.scalar.activation(silu[:], ps_u[:], AF.Silu)
                    nc.vector.tensor_mul(
                        prod8[f // 2][:, (f % 2) * T:(f % 2 + 1) * T],
                        silu[:], ps_w[:])

                for do in range(NCH):
                    if do + 2 < NCH:
                        w2_t[do + 2] = P4.tile([128, F], F8, tag="w2_t",
                                               bufs=2, name=f"w2_{do + 2}")
                        nc.sync.dma_start(w2_t[do + 2][:], w2_8[do + 2])
                    ps_y = PS4.tile([128, T], F32, tag="ps_y", bufs=2,
                                    name=f"ps_y{do}")
                    for fp in range(NFC // 2):
                        fsl = slice(fp * 256, (fp + 1) * 256)
                        nc.tensor.matmul(ps_y[:], dr3(w2_t[do][:, fsl]),
                                         dr3(prod8[fp][:]), start=(fp == 0),
                                         stop=(fp == NFC // 2 - 1),
                                         perf_mode=DR)
                    outt = P4.tile([128, T], F32, tag="outt", bufs=2,
                                   name=f"outt{do}")
                    nc.vector.tensor_add(outt[:], ps_y[:], xloc[do][:])
                    nc.sync.dma_start(outT[do], outt[:])

    nc.compile()
    return nc


def _f8(a):
    return np.clip(a, -240.0, 240.0).astype(ml_dtypes.float8_e4m3)


def prep_inputs(x, mask, wq, wk, wv, wo, w1, w2, w3, g_attn, g_ffn):
    """Build the 8 per-core input maps (host-side sharding + layout)."""
    bf = ml_dtypes.bfloat16

    def dr_w(w, g):
        # [cp, p, (j, m)]: lhsT[p, j, m] = (w*g)[m, cp*256 + j*128 + p]
        wt = (w * g[None, :]).T                      # [d_in, d_out]
        return _f8(np.ascontiguousarray(
            wt.reshape(NCP, 2, 128, D).transpose(0, 2, 1, 3)
            .reshape(NCP, 128, 2 * D)))

    wq8 = dr_w(wq, g_attn)
    wk8 = dr_w(wk, g_attn)
    wv8 = dr_w(wv, g_attn)
    # wo8[pc, p, j, m] = wo[m, (2pc+j)*64 + p]
    wo8 = np.ascontiguousarray(
        wo.T.reshape(H // 2, 2, DK, D).transpose(0, 2, 1, 3)
        .reshape(H // 2, DK, 2 * D)).astype(bf)

    def dr_ffn(w, g):
        # [f, p, (cp, j, m)]: lhsT[p, cp, j, m] = (w*g)[f*128+m, cp*256+j*128+p]
        wt = (w * g[None, :]).T                      # [D, F]
        t = wt.reshape(NCP, 2, 128, NFC, 128).transpose(3, 2, 0, 1, 4)
        return _f8(np.ascontiguousarray(t.reshape(NFC, 128, D)))

    w1_8 = dr_ffn(w1, g_ffn)
    w3_8 = dr_ffn(w3, g_ffn)
    # w2_8[do, p, fp, j, m] = w2[do*128+m, fp*256 + j*128 + p]
    w2_8v = _f8(np.ascontiguousarray(
        w2.T.reshape(NFC // 2, 2, 128, NCH, 128).transpose(3, 2, 0, 1, 4)
        .reshape(NCH, 128, F)))
    ones8 = np.ones((128, 256), ml_dtypes.float8_e4m3)
    ones16 = np.ones((128, 128), bf)

    in_maps = []
    for core in range(8):
        b, qt = core // NQT, core % NQT
        # rotate tokens so the local 512-query slice is always quarter 0
        order = (np.arange(S) + qt * T) % S
        xb = x[b][order]                       # [S, D] rotated
        xTe = np.ascontiguousarray(xb.T.reshape(NCH, 128, S)).astype(np.float32)
        # maskT[p, kt*T + q] = mask[b, qt*T + q, k] with k = kt*128 + p in
        # ROTATED key order (keys follow the same rotation as tokens).
        msl = mask[b, qt * T:(qt + 1) * T][:, order]     # [T(q), S(k)] rotated
        maskTe = np.ascontiguousarray(
            msl.T.reshape(NKT, 128, T).transpose(1, 0, 2)
            .reshape(128, NKT * T)).astype(bf)
        in_maps.append({
            "xT": xTe, "maskT": maskTe,
            "wq8": wq8, "wk8": wk8, "wv8": wv8, "wo8": wo8,
            "w1_8": w1_8, "w3_8": w3_8, "w2_8": w2_8v,
            "ones8": ones8, "ones16": ones16,
        })
    return in_maps


_NC_CACHE = None


def get_nc():
    global _NC_CACHE
    if _NC_CACHE is None:
        _NC_CACHE = build_nc()
    return _NC_CACHE


def gather_output(results):
    out = np.empty((B, S, D), np.float32)
    for core in range(8):
        b, qt = core // NQT, core % NQT
        o = results[core]["outT"]              # [NCH, 128, T]
        out[b, qt * T:(qt + 1) * T, :] = o.reshape(D, T).T
    return out


def kernel(**inputs):
    from concourse.bass_utils import run_bass_kernel_spmd
    in_maps = prep_inputs(
        np.asarray(inputs["x"]), np.asarray(inputs["mask"]),
        np.asarray(inputs["wq"]), np.asarray(inputs["wk"]),
        np.asarray(inputs["wv"]), np.asarray(inputs["wo"]),
        np.asarray(inputs["w1"]), np.asarray(inputs["w2"]),
        np.asarray(inputs["w3"]),
        np.asarray(inputs["g_attn"]), np.asarray(inputs["g_ffn"]))
    nc = get_nc()
    res = run_bass_kernel_spmd(nc, in_maps, core_ids=list(range(8)))
    return gather_output(res.results)
